# revision 20
# baseline (speedup 1.0000x reference)
"""Conformer encoder layer on 8 Trainium2 NeuronCores.

Sharding: pure data-parallel over batch N=16 -> 2 batches/core, no collectives.
Layout: activations kept transposed (features on partitions, time on free dim);
all weights host-pre-transposed to (in_features, out_features).
Precision: fp16 matmul operands (weights + activations) with fp32 PSUM
accumulation; biases/softmax stats/norm in fp32. All weights resident in SBUF.
Depthwise conv as 31 diagonal-matrix matmuls on the PE. Rel-shift via a DRAM
round-trip with a diagonal (stride-639) read access pattern. A^T for the A@V
matmul via DMA xbar transpose.
"""
import sys
sys.path.insert(0, '/opt/trn_rl_repo')
import numpy as np

T, N, E, H, DFF, KC = 512, 16, 512, 8, 2048, 31
D = E // H          # 64
NB = 2              # batches per core
NCORE = 8
PAD = (KC - 1) // 2  # 15

_cached = {}


def pr_of(hh):
    return slice(hh * 64, hh * 64 + 64)


def _build(repeat=1):
    import bass_rust
    import concourse.bass as bass
    import concourse.bacc as bacc
    import concourse.mybir as mybir
    import concourse.tile as tile

    dt = mybir.dt
    Alu = mybir.AluOpType
    Act = mybir.ActivationFunctionType
    ts = bass.ts
    F32, F32R, F16 = dt.float32, dt.float32r, dt.float16

    nc = bacc.Bacc("TRN2", target_bir_lowering=False, debug=False)

    def din(name, shape, dtype=F32):
        return nc.dram_tensor(name, list(shape), dtype,
                              kind="ExternalInput").ap()

    xt_d = din("xt", (NB, E, T), F16)
    pos_d = din("pos_t", (E, 1024), F16)
    w_ffm1_d = din("w_ffm1", (E, DFF), F16)
    bf1_d = din("bf1", (128, 16))
    bf1m_d = din("bf1m", (128, 16))
    w_ffm2_d = din("w_ffm2", (DFF, E), F16)
    bf2_d = din("bf2", (128, 4))
    w_q_d = din("w_q", (E, E), F16)
    w_k_d = din("w_k", (E, E), F16)
    w_v_d = din("w_v", (E, E), F16)
    bq_d = din("bq", (128, 4))
    bk_d = din("bk", (128, 4))
    dvu_d = din("dvu", (128, 4))
    bv_row_d = din("bv_row", (1, E), F16)
    w_pos_d = din("w_pos", (E, E), F16)
    w_out_d = din("w_out", (E, E), F16)
    bo_d = din("bo", (128, 4))
    w_pw1_d = din("w_pw1", (E, 2 * E), F16)
    bpa_d = din("bpa", (128, 4))
    bpb_d = din("bpb", (128, 4))
    w_dwdiag_d = din("w_dwdiag", (128, 4 * KC * 128), F16)
    bdw_d = din("bdw", (128, 4))
    bdwm_d = din("bdwm", (128, 4))
    w_pw2_d = din("w_pw2", (E, E), F16)
    bp2_d = din("bp2", (128, 4))
    w_ff1_d = din("w_ff1", (E, DFF), F16)
    bg1_d = din("bg1", (128, 16))
    bg1m_d = din("bg1m", (128, 16))
    w_ff2_d = din("w_ff2", (DFF, E), F16)
    bg2_d = din("bg2", (128, 4))
    eps_d = din("eps_c", (1, 1))
    ones16_d = din("ones16", (1, 128), F16)
    onescol16_d = din("onescol16", (128, 1), F16)
    ones32_d = din("ones32", (1, 128))
    ident16_d = din("ident16", (128, 128), F16)

    yt_d = nc.dram_tensor("yt", [NB, E, T], F32, kind="ExternalOutput").ap()

    # per-(n, h, tt) rel-shift scratch in DRAM
    bds_d = [[[nc.dram_tensor(f"bds_{n}_{h}_{tt}", [128, 640], F16,
                              kind="Internal").ap()
               for tt in range(4)] for h in range(H)] for n in range(NB)]

    def diag_ap(d_ap):
        # read[tl, j] = flat[tl*639 + 127 + j]  (rel-shift within a t-tile)
        a = d_ap.flatten().copy()
        a.ap = bass_rust.VecI64Pair([[639, 128], [1, 512]])
        a.offset = 127
        return a

    def r3(ap2d):
        # (E-like, F) dram -> (128, a, F) partition view
        return ap2d.rearrange("(a p) f -> p a f", p=128)

    with tile.TileContext(nc) as tc:
        cpool_ctx = tc.tile_pool(name="consts", bufs=1)
        cpool = cpool_ctx.__enter__()
        wts_ctx = tc.tile_pool(name="wts", bufs=1)
        wts = wts_ctx.__enter__()
        xpool_ctx = tc.tile_pool(name="xs", bufs=1)
        xpool = xpool_ctx.__enter__()
        ppool_ctx = tc.tile_pool(name="ptiles", bufs=1)
        ppool = ppool_ctx.__enter__()
        psum_ctx = tc.tile_pool(name="psum", bufs=1, space="PSUM")
        psum = psum_ctx.__enter__()

        def pwork(name):
            return psum.tile([128, 512], F32, tag="work", bufs=4, name=name)

        def pacc(name):
            return psum.tile([128, 512], F32, tag="acc", bufs=4, name=name)

        # ---- constants ----
        def cload(name, d_ap, shape, dtype=F32):
            t_ = cpool.tile(list(shape), dtype, name=name)
            nc.sync.dma_start(t_[:], d_ap if dtype != F32R
                              else d_ap.bitcast(F32R))
            return t_

        bf1_sb = cload("bf1_sb", bf1_d, (128, 16))
        bf1m_sb = cload("bf1m_sb", bf1m_d, (128, 16))
        bf2_sb = cload("bf2_sb", bf2_d, (128, 4))
        bq_sb = cload("bq_sb", bq_d, (128, 4))
        bk_sb = cload("bk_sb", bk_d, (128, 4))
        dvu_sb = cload("dvu_sb", dvu_d, (128, 4))
        bv_row_sb = cload("bv_row_sb", bv_row_d, (1, E), F16)
        bo_sb = cload("bo_sb", bo_d, (128, 4))
        bpa_sb = cload("bpa_sb", bpa_d, (128, 4))
        bpb_sb = cload("bpb_sb", bpb_d, (128, 4))
        bdw_sb = cload("bdw_sb", bdw_d, (128, 4))
        bdwm_sb = cload("bdwm_sb", bdwm_d, (128, 4))
        bp2_sb = cload("bp2_sb", bp2_d, (128, 4))
        bg1_sb = cload("bg1_sb", bg1_d, (128, 16))
        bg1m_sb = cload("bg1m_sb", bg1m_d, (128, 16))
        bg2_sb = cload("bg2_sb", bg2_d, (128, 4))
        eps_sb = cload("eps_sb", eps_d, (1, 1))
        ones16_sb = cload("ones16_sb", ones16_d, (1, 128), F16)
        onescol16_sb = cload("onescol16_sb", onescol16_d, (128, 1), F16)
        ones32r_sb = cload("ones32r_sb", ones32_d, (1, 128), F32R)
        ident16_sb = cload("ident16_sb", ident16_d, (128, 128), F16)

        # ---- resident weights (all fp16), loaded in use-order across
        # both HWDGE queues (SP + ACT) ----
        _dma_alt = [0]

        def wdma(dst, src):
            eng = nc.sync if _dma_alt[0] % 2 == 0 else nc.scalar
            _dma_alt[0] += 1
            eng.dma_start(dst, src)

        def wload(pref, d_ap, fdim, ntile):
            out = []
            rr = r3(d_ap)
            for et in range(ntile):
                wt = wts.tile([128, fdim], F16, name=f"{pref}{et}")
                wdma(wt[:], rr[:, et, :])
                out.append(wt)
            return out

        wpos_sb = wload("wpos_", w_pos_d, E, 4)

        # first-rep inputs: emitted here so their DMAs are not queued behind
        # the bulk weight loads
        def load_inputs():
            xs = []
            for n in range(NB):
                x0 = xpool.tile([128, 4, 512], F16, tag=f"x{n}", bufs=2,
                                name=f"x0_{n}")
                nc.scalar.dma_start(x0[:], r3(xt_d[n]))
                xs.append(x0)
            ps, ps_free = tc.tile([128, 4, 1024], F16, name="pos_sb")
            nc.scalar.dma_start(ps[:], r3(pos_d))
            return xs, ps, ps_free

        first_inputs = [load_inputs()]

        wq_sb = wload("wq_", w_q_d, E, 4)
        wk_sb = wload("wk_", w_k_d, E, 4)
        wv_sb = wload("wv_", w_v_d, E, 4)
        wo_sb = wload("wo_", w_out_d, E, 4)

        # ---- per-repetition body (repeat>1 used for HW timing) ----
        def emit_rep():
            def xtile(n, stage):
                return xpool.tile([128, 4, 512], F16, tag=f"x{n}", bufs=2,
                                  name=f"x{stage}_{n}")

            if first_inputs:
                x_cur, pos_sb, pos_free = first_inputs.pop()
            else:
                x_cur = []
                for n in range(NB):
                    x0 = xtile(n, 0)
                    nc.scalar.dma_start(x0[:], r3(xt_d[n]))
                    x_cur.append(x0)
                pos_sb, pos_free = tc.tile([128, 4, 1024], F16,
                                           name="pos_sb")
                nc.scalar.dma_start(pos_sb[:], r3(pos_d))


            # ---- FFN (macaron + final) ----
            def ffn(tag, w1_d, b1, b1m, w2_d, b2, stage):
                with tc.tile_pool(name=f"s{tag}", bufs=1) as wp:
                    w1_sb = []
                    w1_r = r3(w1_d)
                    for et in range(4):
                        wt = wp.tile([128, DFF], F16, name=f"{tag}w1_{et}")
                        wdma(wt[:], w1_r[:, et, :])
                        w1_sb.append(wt)
                    w2_sb = []
                    w2_r = r3(w2_d)
                    for dtl in range(16):
                        wt = wp.tile([128, E], F16, name=f"{tag}w2_{dtl}")
                        wdma(wt[:], w2_r[:, dtl, :])
                        w2_sb.append(wt)
                    x_new = []
                    for n in range(NB):
                        xin = x_cur[n]
                        accs = [pacc(f"{tag}acc{n}_{et}") for et in range(4)]
                        sds = []

                        def h2_emit(d):
                            for et in range(4):
                                nc.tensor.matmul(
                                    accs[et][:], w2_sb[d][:, ts(et, 128)],
                                    sds[d][:], start=(d == 0), stop=(d == 15))

                        for d in range(16):
                            hps = pwork(f"{tag}h1_{n}_{d}")
                            for et in range(4):
                                nc.tensor.matmul(
                                    hps[:], w1_sb[et][:, ts(d, 128)],
                                    xin[:, et, :],
                                    start=(et == 0), stop=(et == 3))
                            sg = wp.tile([128, 512], F32, tag="ffsg", bufs=3,
                                         name=f"{tag}sg{n}{d}")
                            nc.scalar.activation(sg[:], hps[:], Act.Sigmoid,
                                                 bias=b1m[:, d:d + 1])
                            sd = wp.tile([128, 512], F16, tag="ffsd", bufs=4,
                                         name=f"{tag}sd{n}{d}")
                            nc.vector.scalar_tensor_tensor(
                                sd[:], hps[:], b1[:, d:d + 1], sg[:],
                                op0=Alu.add, op1=Alu.mult)
                            sds.append(sd)
                            if d >= 1:
                                h2_emit(d - 1)
                        h2_emit(15)
                        xo = xtile(n, stage)
                        for et in range(4):
                            nc.vector.scalar_tensor_tensor(
                                xo[:, et, :], accs[et][:], b2[:, et:et + 1],
                                xin[:, et, :], op0=Alu.add, op1=Alu.add)
                        x_new.append(xo)
                    for n in range(NB):
                        x_cur[n] = x_new[n]

            ffn("ffm", w_ffm1_d, bf1_sb, bf1m_sb, w_ffm2_d, bf2_sb, 1)

            # p^T = pos_w @ pos_emb^T
            pT_sb = ppool.tile([128, 4, 1024], F16, tag="pT", name="pT_sb")
            for pf in range(4):
                for half in range(2):
                    ps = pwork(f"pps{pf}{half}")
                    for et in range(4):
                        nc.tensor.matmul(ps[:], wpos_sb[et][:, ts(pf, 128)],
                                         pos_sb[:, et, ts(half, 512)],
                                         start=(et == 0), stop=(et == 3))
                    nc.vector.tensor_copy(pT_sb[:, pf, ts(half, 512)], ps[:])
            pos_free()

            # ---- attention (head-pairs interleaved across batches) ----
            with tc.tile_pool(name="sattn", bufs=1) as wa:
                q_sb, k_sb, qv_sb, v_sb, oT_sb = [], [], [], [], []
                for n in range(NB):
                    x1 = x_cur[n]
                    q_ = wa.tile([128, 4, 512], F16, tag=f"q{n}",
                                 name=f"q_{n}")
                    k_ = wa.tile([128, 4, 512], F16, tag=f"k{n}",
                                 name=f"k_{n}")
                    qv_ = wa.tile([128, 4, 512], F16, tag=f"qv{n}",
                                  name=f"qv_{n}")
                    v_ = wa.tile([128, 4, 512], F16, tag=f"v{n}",
                                 name=f"v_{n}")
                    for i in range(4):
                        qps = pwork(f"qps{n}{i}")
                        for et in range(4):
                            nc.tensor.matmul(qps[:], wq_sb[et][:, ts(i, 128)],
                                             x1[:, et, :],
                                             start=(et == 0), stop=(et == 3))
                        nc.scalar.activation(q_[:, i, :], qps[:],
                                             Act.Identity,
                                             bias=bq_sb[:, i:i + 1])
                        kps = pwork(f"kps{n}{i}")
                        for et in range(4):
                            nc.tensor.matmul(kps[:], wk_sb[et][:, ts(i, 128)],
                                             x1[:, et, :],
                                             start=(et == 0), stop=(et == 3))
                        nc.scalar.activation(k_[:, i, :], kps[:],
                                             Act.Identity,
                                             bias=bk_sb[:, i:i + 1])
                        nc.gpsimd.tensor_scalar_add(qv_[:, i, :],
                                                    q_[:, i, :],
                                                    dvu_sb[:, i:i + 1])
                    for tt in range(4):
                        vps = pwork(f"vps{n}{tt}")
                        for et in range(4):
                            nc.tensor.matmul(vps[:], x1[:, et, ts(tt, 128)],
                                             wv_sb[et][:], start=(et == 0),
                                             stop=False)
                        nc.tensor.matmul(vps[:], ones16_sb[:], bv_row_sb[:],
                                         start=False, stop=True)
                        nc.vector.tensor_copy(v_[:, tt, :], vps[:])
                    q_sb.append(q_)
                    k_sb.append(k_)
                    qv_sb.append(qv_)
                    v_sb.append(v_)
                    oT_sb.append(wa.tile([128, 4, 512], F16, tag=f"oT{n}",
                                         name=f"oT_{n}"))

                LAG = 6
                units = [(hp, n, hh, tt) for hp in range(4)
                         for n in range(NB) for hh in range(2)
                         for tt in range(4)]
                at_tiles = {}
                bdsh_t = {}
                av_pend = []

                def produce(u):
                    hp, n, hh, tt = u
                    h = 2 * hp + hh
                    pr = slice(hh * 64, hh * 64 + 64)
                    tpos = (hh * 64, 0)
                    w0 = 384 - tt * 128
                    bdA = pwork(f"bdA{n}{h}{tt}")
                    bdB = psum.tile([128, 128], F32, tag="work",
                                    bufs=4, name=f"bdB{n}{h}{tt}")
                    nc.tensor.matmul(
                        bdA[:], qv_sb[n][pr, hp, ts(tt, 128)],
                        pT_sb[pr, hp, w0:w0 + 512],
                        start=True, stop=True, tile_position=tpos)
                    nc.tensor.matmul(
                        bdB[:], qv_sb[n][pr, hp, ts(tt, 128)],
                        pT_sb[pr, hp, w0 + 512:w0 + 640],
                        start=True, stop=True, tile_position=tpos)
                    bd_sb = wa.tile([128, 640], F16, tag="bdsb",
                                    bufs=8, name=f"bdsb{n}{h}{tt}")
                    nc.vector.tensor_copy(bd_sb[:, 0:512], bdA[:])
                    nc.scalar.copy(bd_sb[:, 512:640], bdB[:])
                    nc.scalar.dma_start(bds_d[n][h][tt], bd_sb[:])
                    bdsh = wa.tile([128, 512], F16, tag="bdsh",
                                   bufs=8, name=f"bdsh{n}{h}{tt}")
                    nc.scalar.dma_start(bdsh[:], diag_ap(bds_d[n][h][tt]))
                    bdsh_t[u] = bdsh

                soft_pend = []

                def consume(u):
                    hp, n, hh, tt = u
                    h = 2 * hp + hh
                    pr = slice(hh * 64, hh * 64 + 64)
                    tpos = (hh * 64, 0)
                    acps = pacc(f"ac{n}{h}{tt}")
                    nc.tensor.matmul(
                        acps[:], q_sb[n][pr, hp, ts(tt, 128)],
                        k_sb[n][pr, hp, :],
                        start=True, stop=True, tile_position=tpos)
                    sc = wa.tile([128, 512], F32, tag="sc",
                                 bufs=6, name=f"sc{n}{h}{tt}")
                    nc.vector.tensor_add(sc[:], acps[:], bdsh_t.pop(u)[:])
                    e_t = wa.tile([128, 512], F16, tag="esb",
                                  bufs=8, name=f"e{n}{h}{tt}")
                    zz = wa.tile([128, 1], F32, tag="z", bufs=8,
                                 name=f"z{n}{h}{tt}")
                    nc.scalar.activation(e_t[:], sc[:], Act.Exp,
                                         accum_out=zz[:])
                    soft_pend.append((u, e_t, zz))
                    if len(soft_pend) > 1:
                        consume_b(*soft_pend.pop(0))

                def consume_b(u, e_t, zz):
                    hp, n, hh, tt = u
                    h = 2 * hp + hh
                    rz = wa.tile([128, 1], F32, tag="rz", bufs=8,
                                 name=f"rz{n}{h}{tt}")
                    nc.vector.reciprocal(rz[:], zz[:])
                    a_t = wa.tile([128, 512], F16, tag="asb",
                                  bufs=8, name=f"a{n}{h}{tt}")
                    nc.gpsimd.tensor_scalar_mul(a_t[:], e_t[:], rz[:, 0:1])
                    if tt == 0:
                        at_tiles[(n, hp, hh)] = wa.tile(
                            [128, 4, 512], F16, tag="at", bufs=6,
                            name=f"at{n}{hp}{hh}")
                    trp = psum.tile([128, 512], F16, tag="work", bufs=4,
                                    name=f"trp{n}{h}{tt}")
                    for i_ in range(4):
                        nc.tensor.matmul(trp[:, ts(i_, 128)],
                                         a_t[:, ts(i_, 128)], ident16_sb[:],
                                         is_transpose=True, start=True,
                                         stop=True, skip_group_check=True)
                    nc.vector.tensor_copy(
                        at_tiles[(n, hp, hh)][:, :, ts(tt, 128)],
                        trp[:].rearrange("p (a b) -> p a b", a=4))
                    if tt == 3:
                        av_pend.append((hp, n, hh))

                def emit_av(key):
                    hp, n, hh = key
                    h = 2 * hp + hh
                    ats = at_tiles.pop((n, hp, hh))
                    ops_ = psum.tile([64, 512], F32, tag="acc",
                                     bufs=4, name=f"ops{n}{h}")
                    for st in range(4):
                        nc.tensor.matmul(
                            ops_[:], v_sb[n][:, st, h * 64:h * 64 + 64],
                            ats[:, st, :], start=(st == 0), stop=(st == 3))
                    nc.vector.tensor_copy(oT_sb[n][pr_of(hh), hp, :], ops_[:])
                    if hp == 3 and hh == 1:
                        oproj(n)

                def oproj(n):
                    x2 = xtile(n, 2)
                    for of in range(4):
                        pps = pwork(f"oproj{n}{of}")
                        for hp in range(4):
                            nc.tensor.matmul(pps[:], wo_sb[hp][:, ts(of, 128)],
                                             oT_sb[n][:, hp, :],
                                             start=(hp == 0), stop=(hp == 3))
                        nc.vector.scalar_tensor_tensor(
                            x2[:, of, :], pps[:], bo_sb[:, of:of + 1],
                            x_cur[n][:, of, :], op0=Alu.add, op1=Alu.add)
                    x_cur[n] = x2

                for i, u in enumerate(units):
                    produce(u)
                    if i >= LAG:
                        consume(units[i - LAG])
                        if len(av_pend) > 2:
                            emit_av(av_pend.pop(0))
                for i in range(len(units) - LAG, len(units)):
                    consume(units[i])
                    if len(av_pend) > 2:
                        emit_av(av_pend.pop(0))
                while soft_pend:
                    consume_b(*soft_pend.pop(0))
                    if len(av_pend) > 2:
                        emit_av(av_pend.pop(0))
                while av_pend:
                    emit_av(av_pend.pop(0))

            # ---- conv module ----
            with tc.tile_pool(name="sconv", bufs=1) as wc:
                wpw1_sb = []
                pw1_r = r3(w_pw1_d)
                for et in range(4):
                    wt = wc.tile([128, 2 * E], F16, name=f"wpw1_{et}")
                    nc.sync.dma_start(wt[:], pw1_r[:, et, :])
                    wpw1_sb.append(wt)
                dwdiag_sb = wc.tile([128, 4, KC, 128], F16, name="dwdiag_sb")
                nc.scalar.dma_start(
                    dwdiag_sb[:],
                    w_dwdiag_d.rearrange("p (c k j) -> p c k j", c=4, k=KC))
                wpw2_sb = []
                pw2_r = r3(w_pw2_d)
                for et in range(4):
                    wt = wc.tile([128, E], F16, name=f"wpw2_{et}")
                    nc.sync.dma_start(wt[:], pw2_r[:, et, :])
                    wpw2_sb.append(wt)
                for n in range(NB):
                    x2 = x_cur[n]
                    ys = []
                    for cf in range(4):
                        bps = pwork(f"glb{n}{cf}")
                        for et in range(4):
                            nc.tensor.matmul(bps[:],
                                             wpw1_sb[et][:, ts(cf + 4, 128)],
                                             x2[:, et, :],
                                             start=(et == 0), stop=(et == 3))
                        sgl = wc.tile([128, 512], F32, tag="cvsg", bufs=3,
                                      name=f"cvsg{n}{cf}")
                        nc.scalar.activation(sgl[:], bps[:], Act.Sigmoid,
                                             bias=bpb_sb[:, cf:cf + 1])
                        aps = pwork(f"gla{n}{cf}")
                        for et in range(4):
                            nc.tensor.matmul(aps[:],
                                             wpw1_sb[et][:, ts(cf, 128)],
                                             x2[:, et, :],
                                             start=(et == 0), stop=(et == 3))
                        glu = wc.tile([128, 542], F16, tag="glu", bufs=6,
                                      name=f"glu{n}{cf}")
                        nc.gpsimd.memset(glu[:, 0:PAD], 0.0)
                        nc.gpsimd.memset(glu[:, 527:542], 0.0)
                        nc.vector.scalar_tensor_tensor(
                            glu[:, PAD:527], aps[:], bpa_sb[:, cf:cf + 1],
                            sgl[:], op0=Alu.add, op1=Alu.mult)
                        # depthwise conv: 31 diagonal-matrix matmuls on PE
                        dwps = pacc(f"dwps{n}{cf}")
                        for k_ in range(KC):
                            nc.tensor.matmul(dwps[:], dwdiag_sb[:, cf, k_, :],
                                             glu[:, k_:k_ + 512],
                                             start=(k_ == 0),
                                             stop=(k_ == KC - 1))
                        sg2 = wc.tile([128, 512], F32, tag="cvsg", bufs=3,
                                      name=f"dwsg{n}{cf}")
                        nc.scalar.activation(sg2[:], dwps[:], Act.Sigmoid,
                                             bias=bdwm_sb[:, cf:cf + 1])
                        y_ = wc.tile([128, 512], F16, tag="ydw", bufs=5,
                                     name=f"ydw{n}{cf}")
                        nc.vector.scalar_tensor_tensor(
                            y_[:], dwps[:], bdw_sb[:, cf:cf + 1], sg2[:],
                            op0=Alu.add, op1=Alu.mult)
                        ys.append(y_)
                    x3 = xtile(n, 3)
                    for of in range(4):
                        cps = pacc(f"pw2{n}{of}")
                        for cf in range(4):
                            nc.tensor.matmul(cps[:],
                                             wpw2_sb[cf][:, ts(of, 128)],
                                             ys[cf][:],
                                             start=(cf == 0), stop=(cf == 3))
                        nc.vector.scalar_tensor_tensor(
                            x3[:, of, :], cps[:], bp2_sb[:, of:of + 1],
                            x2[:, of, :], op0=Alu.add, op1=Alu.add)
                    x_cur[n] = x3

            # ---- final FFN ----
            ffn("ff2", w_ff1_d, bg1_sb, bg1m_sb, w_ff2_d, bg2_sb, 4)

            # ---- BasicNorm + output ----
            yt_r = [r3(yt_d[n]) for n in range(NB)]
            with tc.tile_pool(name="nrm", bufs=1) as nrm:
                for n in range(NB):
                    x4 = x_cur[n]
                    msps = psum.tile([1, 512], F32, tag="work", bufs=4,
                                     name=f"ms{n}")
                    for et in range(4):
                        sq = nrm.tile([128, 512], F16, tag="sq", bufs=2,
                                      name=f"sq{n}{et}")
                        nc.scalar.activation(sq[:], x4[:, et, :], Act.Square)
                        nc.tensor.matmul(msps[:], onescol16_sb[:], sq[:],
                                         start=(et == 0), stop=(et == 3))
                    sc1 = nrm.tile([1, 512], F32, tag="sc1", bufs=2,
                                   name=f"sc1{n}")
                    nc.scalar.activation(sc1[:], msps[:], Act.Sqrt,
                                         bias=eps_sb[0:1, 0:1], scale=1.0 / E)
                    rsc = nrm.tile([1, 512], F32, tag="rsc", bufs=2,
                                   name=f"rsc{n}")
                    nc.vector.reciprocal(rsc[:], sc1[:])
                    rscr = nrm.tile([1, 512], F32R, tag="rscr", bufs=2,
                                    name=f"rscr{n}")
                    nc.vector.tensor_copy(rscr[:], rsc[:])
                    bcps = pacc(f"bc{n}")
                    nc.tensor.matmul(bcps[:], ones32r_sb[:], rscr[:],
                                     start=True, stop=True)
                    for et in range(4):
                        yo = nrm.tile([128, 512], F32, tag="yo", bufs=3,
                                      name=f"yo{n}{et}")
                        nc.vector.tensor_mul(yo[:], x4[:, et, :], bcps[:])
                        nc.sync.dma_start(yt_r[n][:, et, :], yo[:])

        for _rep in range(repeat):
            emit_rep()

        psum_ctx.__exit__(None, None, None)
        ppool_ctx.__exit__(None, None, None)
        xpool_ctx.__exit__(None, None, None)
        wts_ctx.__exit__(None, None, None)
        cpool_ctx.__exit__(None, None, None)

    nc.compile()
    return nc


def _prep_inputs(inputs):
    f32 = np.float32
    f16 = np.float16
    s = np.float32(D ** -0.5)
    src = np.asarray(inputs['src'], f32)
    pos_emb = np.asarray(inputs['pos_emb'], f32)
    ipw = np.asarray(inputs['in_proj_w'], f32)
    ipb = np.asarray(inputs['in_proj_b'], f32)
    bu = np.asarray(inputs['pos_bias_u'], f32).reshape(E)
    bv = np.asarray(inputs['pos_bias_v'], f32).reshape(E)

    def t_(a):
        return np.ascontiguousarray(np.asarray(a, f32).T.astype(f16))

    def btile(b):  # (F,) -> (128, F//128) with [p, i] = b[i*128+p]
        b = np.asarray(b, f32)
        return np.ascontiguousarray(b.reshape(-1, 128).T)

    pos_t = np.zeros((E, 1024), f16)
    pos_t[:, :2 * T - 1] = pos_emb[0].T.astype(f16)

    dw = np.asarray(inputs['conv_dw_w'], f32).reshape(E, KC)
    dwr = dw.reshape(4, 128, KC).transpose(1, 0, 2)      # (128p, 4cf, 31k)
    dwdiag = np.zeros((128, 4, KC, 128), f16)
    pidx = np.arange(128)
    dwdiag[pidx, :, :, pidx] = dwr.astype(f16)
    w_dwdiag = np.ascontiguousarray(dwdiag.reshape(128, 4 * KC * 128))

    common = {
        'pos_t': pos_t,
        'w_ffm1': t_(inputs['ffm_w1']), 'bf1': btile(inputs['ffm_b1']),
        'bf1m': btile(np.asarray(inputs['ffm_b1'], f32) - 1.0),
        'w_ffm2': t_(inputs['ffm_w2']), 'bf2': btile(inputs['ffm_b2']),
        'w_q': np.ascontiguousarray((ipw[0:E] * s).T.astype(f16)),
        'w_k': t_(ipw[E:2 * E]), 'w_v': t_(ipw[2 * E:3 * E]),
        'bq': btile(ipb[0:E] * s + bu), 'bk': btile(ipb[E:2 * E]),
        'dvu': btile(bv - bu),
        'bv_row': np.ascontiguousarray(
            ipb[2 * E:3 * E].reshape(1, E).astype(f16)),
        'w_pos': t_(inputs['pos_w']),
        'w_out': t_(inputs['out_w']), 'bo': btile(inputs['out_b']),
        'w_pw1': t_(inputs['conv_pw1_w']),
        'bpa': btile(np.asarray(inputs['conv_pw1_b'], f32)[0:E]),
        'bpb': btile(np.asarray(inputs['conv_pw1_b'], f32)[E:2 * E]),
        'w_dwdiag': w_dwdiag, 'bdw': btile(inputs['conv_dw_b']),
        'bdwm': btile(np.asarray(inputs['conv_dw_b'], f32) - 1.0),
        'w_pw2': t_(inputs['conv_pw2_w']), 'bp2': btile(inputs['conv_pw2_b']),
        'w_ff1': t_(inputs['ff_w1']), 'bg1': btile(inputs['ff_b1']),
        'bg1m': btile(np.asarray(inputs['ff_b1'], f32) - 1.0),
        'w_ff2': t_(inputs['ff_w2']), 'bg2': btile(inputs['ff_b2']),
        'eps_c': np.exp(np.asarray(inputs['norm_eps'], f32)).reshape(1, 1),
        'ones16': np.ones((1, 128), f16),
        'onescol16': np.ones((128, 1), f16),
        'ones32': np.ones((1, 128), f32),
        'ident16': np.eye(128, dtype=f16),
    }

    src_t = np.ascontiguousarray(src.transpose(1, 2, 0))  # (N, E, T)
    in_maps = []
    for c in range(NCORE):
        m = dict(common)
        m['xt'] = np.ascontiguousarray(
            src_t[NB * c:NB * (c + 1)].astype(f16))
        in_maps.append(m)
    return in_maps


def _run(inputs, trace=False):
    from concourse import bass_utils
    if 'nc1' not in _cached:
        _cached['nc1'] = _build()
    nc = _cached['nc1']
    in_maps = _prep_inputs(inputs)
    res = bass_utils.run_bass_kernel_spmd(nc, in_maps,
                                          core_ids=list(range(NCORE)),
                                          trace=trace)
    yts = np.stack([res.results[c]['yt'] for c in range(NCORE)])  # (8,2,E,T)
    out = np.ascontiguousarray(
        yts.transpose(3, 0, 1, 2).reshape(T, N, E)).astype(np.float32)
    return out, res


def kernel(**inputs):
    out, _ = _run(inputs, trace=False)
    return out


def _make_runner(inputs, repeat=1):
    """Build a zero-transfer on-device runner for timing.

    Mirrors bass2jax.run_bass_via_pjrt's shard_map setup but without buffer
    donation, so nothing is re-transferred between timed calls.
    """
    import jax
    import numpy as _np
    import concourse.mybir as mybir
    from concourse.bass2jax import (_bass_exec_p, install_neuronx_cc_hook,
                                    partition_id_tensor)
    from jax.experimental.shard_map import shard_map
    from jax.sharding import Mesh, PartitionSpec, NamedSharding

    key = f'nc{repeat}'
    if key not in _cached:
        _cached[key] = _build(repeat)
    nc = _cached[key]
    install_neuronx_cc_hook()
    in_maps = _prep_inputs(inputs)

    in_names, out_names, out_avals, zero_outs = [], [], [], []
    for alloc in nc.m.functions[0].allocations:
        if not isinstance(alloc, mybir.MemoryLocationSet):
            continue
        name = alloc.memorylocations[0].name
        if alloc.kind == "ExternalInput":
            if nc.partition_id_tensor is None or \
                    name != nc.partition_id_tensor.name:
                in_names.append(name)
        elif alloc.kind == "ExternalOutput":
            out_names.append(name)
            shape = tuple(alloc.tensor_shape)
            dtype = mybir.dt.np(alloc.dtype)
            out_avals.append(jax.core.ShapedArray(shape, dtype))
            zero_outs.append(_np.zeros(shape, dtype))
    n_params = len(in_names)
    all_names = in_names + out_names
    if nc.partition_id_tensor is not None:
        all_names = all_names + [nc.partition_id_tensor.name]

    def _body(*args):
        operands = list(args)
        if nc.partition_id_tensor is not None:
            operands.append(partition_id_tensor())
        outs = _bass_exec_p.bind(
            *operands, out_avals=tuple(out_avals), in_names=tuple(all_names),
            out_names=tuple(out_names), lowering_input_output_aliases=(),
            sim_require_finite=True, sim_require_nnan=True, nc=nc)
        return tuple(outs)

    devices = jax.devices()[:NCORE]
    mesh = Mesh(_np.asarray(devices), ("core",))
    spec = PartitionSpec("core")
    sharded = jax.jit(shard_map(
        _body, mesh=mesh, in_specs=(spec,) * (n_params + len(out_names)),
        out_specs=(spec,) * len(out_names), check_rep=False))
    sh = NamedSharding(mesh, spec)
    concat_in = [jax.device_put(
        _np.concatenate([_np.asarray(in_maps[c][nm]) for c in range(NCORE)],
                        axis=0), sh) for nm in in_names]
    concat_zero = [jax.device_put(
        _np.zeros((NCORE * z.shape[0], *z.shape[1:]), z.dtype), sh)
        for z in zero_outs]

    def run():
        out = sharded(*concat_in, *concat_zero)
        jax.block_until_ready(out)
        return out

    def gather(out):
        yts = _np.asarray(out[out_names.index('yt')]).reshape(
            NCORE, NB, E, T)
        return _np.ascontiguousarray(
            yts.transpose(3, 0, 1, 2).reshape(T, N, E)).astype(_np.float32)

    return run, gather


def _bench(inputs, iters=10, repeat=1):
    import time
    run, gather = _make_runner(inputs, repeat)
    out = run()
    times = []
    for _ in range(iters):
        t0 = time.perf_counter()
        out = run()
        times.append(time.perf_counter() - t0)
    return gather(out), times


# revision 21
# speedup vs baseline: 1.0583x; 1.0583x over previous
"""Conformer encoder layer on 8 Trainium2 NeuronCores.

Sharding: pure data-parallel over batch N=16 -> 2 batches/core, no collectives.
Layout: activations kept transposed (features on partitions, time on free dim);
all weights host-pre-transposed to (in_features, out_features).
Precision: fp16 matmul operands (weights + activations) with fp32 PSUM
accumulation; biases/softmax stats/norm in fp32. All weights resident in SBUF.
Depthwise conv as 31 diagonal-matrix matmuls on the PE. Rel-shift via a DRAM
round-trip with a diagonal (stride-639) read access pattern. A^T for the A@V
matmul via DMA xbar transpose.
"""
import sys
sys.path.insert(0, '/opt/trn_rl_repo')
import numpy as np

T, N, E, H, DFF, KC = 512, 16, 512, 8, 2048, 31
D = E // H          # 64
NB = 2              # batches per core
NCORE = 8
PAD = (KC - 1) // 2  # 15

_cached = {}


def pr_of(hh):
    return slice(hh * 64, hh * 64 + 64)


def _build(repeat=1):
    import bass_rust
    import concourse.bass as bass
    import concourse.bacc as bacc
    import concourse.mybir as mybir
    import concourse.tile as tile

    dt = mybir.dt
    Alu = mybir.AluOpType
    Act = mybir.ActivationFunctionType
    ts = bass.ts
    F32, F32R, F16 = dt.float32, dt.float32r, dt.float16

    nc = bacc.Bacc("TRN2", target_bir_lowering=False, debug=False)

    def din(name, shape, dtype=F32):
        return nc.dram_tensor(name, list(shape), dtype,
                              kind="ExternalInput").ap()

    xt_d = din("xt", (NB, E, T), F16)
    pos_d = din("pos_t", (E, 1024), F16)
    w_ffm1_d = din("w_ffm1", (E, DFF), F16)
    bf1_d = din("bf1", (128, 16))
    bf1m_d = din("bf1m", (128, 16))
    w_ffm2_d = din("w_ffm2", (DFF, E), F16)
    bf2_d = din("bf2", (128, 4))
    w_q_d = din("w_q", (E, E), F16)
    w_k_d = din("w_k", (E, E), F16)
    w_v_d = din("w_v", (E, E), F16)
    bq_d = din("bq", (128, 4))
    bk_d = din("bk", (128, 4))
    dvu_d = din("dvu", (128, 4))
    bv_row_d = din("bv_row", (1, E), F16)
    w_pos_d = din("w_pos", (E, E), F16)
    w_out_d = din("w_out", (E, E), F16)
    bo_d = din("bo", (128, 4))
    w_pw1_d = din("w_pw1", (E, 2 * E), F16)
    bpa_d = din("bpa", (128, 4))
    bpb_d = din("bpb", (128, 4))
    w_dwdiag_d = din("w_dwdiag", (128, 4 * KC * 128), F16)
    bdw_d = din("bdw", (128, 4))
    bdwm_d = din("bdwm", (128, 4))
    w_pw2_d = din("w_pw2", (E, E), F16)
    bp2_d = din("bp2", (128, 4))
    w_ff1_d = din("w_ff1", (E, DFF), F16)
    bg1_d = din("bg1", (128, 16))
    bg1m_d = din("bg1m", (128, 16))
    w_ff2_d = din("w_ff2", (DFF, E), F16)
    bg2_d = din("bg2", (128, 4))
    eps_d = din("eps_c", (1, 1))
    ones16_d = din("ones16", (1, 128), F16)
    onescol16_d = din("onescol16", (128, 1), F16)
    ones32_d = din("ones32", (1, 128))
    ident16_d = din("ident16", (128, 128), F16)

    yt_d = nc.dram_tensor("yt", [NB, E, T], F32, kind="ExternalOutput").ap()

    # per-(n, h, tt) rel-shift scratch in DRAM
    bds_d = [[[nc.dram_tensor(f"bds_{n}_{h}_{tt}", [128, 640], F16,
                              kind="Internal").ap()
               for tt in range(4)] for h in range(H)] for n in range(NB)]

    def diag_ap(d_ap):
        # read[tl, j] = flat[tl*639 + 127 + j]  (rel-shift within a t-tile)
        a = d_ap.flatten().copy()
        a.ap = bass_rust.VecI64Pair([[639, 128], [1, 512]])
        a.offset = 127
        return a

    def r3(ap2d):
        # (E-like, F) dram -> (128, a, F) partition view
        return ap2d.rearrange("(a p) f -> p a f", p=128)

    with tile.TileContext(nc) as tc:
        cpool_ctx = tc.tile_pool(name="consts", bufs=1)
        cpool = cpool_ctx.__enter__()
        wts_ctx = tc.tile_pool(name="wts", bufs=1)
        wts = wts_ctx.__enter__()
        xpool_ctx = tc.tile_pool(name="xs", bufs=1)
        xpool = xpool_ctx.__enter__()
        ppool_ctx = tc.tile_pool(name="ptiles", bufs=1)
        ppool = ppool_ctx.__enter__()
        psum_ctx = tc.tile_pool(name="psum", bufs=1, space="PSUM")
        psum = psum_ctx.__enter__()

        def pwork(name):
            return psum.tile([128, 512], F32, tag="work", bufs=4, name=name)

        def pacc(name):
            return psum.tile([128, 512], F32, tag="acc", bufs=4, name=name)

        # ---- constants ----
        def cload(name, d_ap, shape, dtype=F32):
            t_ = cpool.tile(list(shape), dtype, name=name)
            nc.sync.dma_start(t_[:], d_ap if dtype != F32R
                              else d_ap.bitcast(F32R))
            return t_

        bf1_sb = cload("bf1_sb", bf1_d, (128, 16))
        bf1m_sb = cload("bf1m_sb", bf1m_d, (128, 16))
        bf2_sb = cload("bf2_sb", bf2_d, (128, 4))
        bq_sb = cload("bq_sb", bq_d, (128, 4))
        bk_sb = cload("bk_sb", bk_d, (128, 4))
        dvu_sb = cload("dvu_sb", dvu_d, (128, 4))
        bv_row_sb = cload("bv_row_sb", bv_row_d, (1, E), F16)
        bo_sb = cload("bo_sb", bo_d, (128, 4))
        bpa_sb = cload("bpa_sb", bpa_d, (128, 4))
        bpb_sb = cload("bpb_sb", bpb_d, (128, 4))
        bdw_sb = cload("bdw_sb", bdw_d, (128, 4))
        bdwm_sb = cload("bdwm_sb", bdwm_d, (128, 4))
        bp2_sb = cload("bp2_sb", bp2_d, (128, 4))
        bg1_sb = cload("bg1_sb", bg1_d, (128, 16))
        bg1m_sb = cload("bg1m_sb", bg1m_d, (128, 16))
        bg2_sb = cload("bg2_sb", bg2_d, (128, 4))
        eps_sb = cload("eps_sb", eps_d, (1, 1))
        ones16_sb = cload("ones16_sb", ones16_d, (1, 128), F16)
        onescol16_sb = cload("onescol16_sb", onescol16_d, (128, 1), F16)
        ones32r_sb = cload("ones32r_sb", ones32_d, (1, 128), F32R)
        ident16_sb = cload("ident16_sb", ident16_d, (128, 128), F16)

        # ---- resident weights (all fp16), loaded in use-order across
        # both HWDGE queues (SP + ACT) ----
        _dma_alt = [0]

        def wdma(dst, src):
            eng = nc.sync if _dma_alt[0] % 2 == 0 else nc.scalar
            _dma_alt[0] += 1
            eng.dma_start(dst, src)

        def wload(pref, d_ap, fdim, ntile):
            out = []
            rr = r3(d_ap)
            for et in range(ntile):
                wt = wts.tile([128, fdim], F16, name=f"{pref}{et}")
                wdma(wt[:], rr[:, et, :])
                out.append(wt)
            return out

        wpos_sb = wload("wpos_", w_pos_d, E, 4)

        # first-rep inputs: emitted here so their DMAs are not queued behind
        # the bulk weight loads
        def load_inputs():
            xs = []
            for n in range(NB):
                x0 = xpool.tile([128, 4, 512], F16, tag=f"x{n}", bufs=2,
                                name=f"x0_{n}")
                nc.scalar.dma_start(x0[:], r3(xt_d[n]))
                xs.append(x0)
            ps, ps_free = tc.tile([128, 4, 1024], F16, name="pos_sb")
            nc.scalar.dma_start(ps[:], r3(pos_d))
            return xs, ps, ps_free

        first_inputs = [load_inputs()]

        wq_sb = wload("wq_", w_q_d, E, 4)
        wk_sb = wload("wk_", w_k_d, E, 4)
        wv_sb = wload("wv_", w_v_d, E, 4)
        wo_sb = wload("wo_", w_out_d, E, 4)

        # ---- per-repetition body (repeat>1 used for HW timing) ----
        def emit_rep():
            def xtile(n, stage):
                return xpool.tile([128, 4, 512], F16, tag=f"x{n}", bufs=2,
                                  name=f"x{stage}_{n}")

            if first_inputs:
                x_cur, pos_sb, pos_free = first_inputs.pop()
            else:
                x_cur = []
                for n in range(NB):
                    x0 = xtile(n, 0)
                    nc.scalar.dma_start(x0[:], r3(xt_d[n]))
                    x_cur.append(x0)
                pos_sb, pos_free = tc.tile([128, 4, 1024], F16,
                                           name="pos_sb")
                nc.scalar.dma_start(pos_sb[:], r3(pos_d))


            # ---- FFN (macaron + final) ----
            def ffn(tag, w1_d, b1, b1m, w2_d, b2, stage):
                with tc.tile_pool(name=f"s{tag}", bufs=1) as wp:
                    w1_sb = []
                    w1_r = r3(w1_d)
                    for et in range(4):
                        wt = wp.tile([128, DFF], F16, name=f"{tag}w1_{et}")
                        wdma(wt[:], w1_r[:, et, :])
                        w1_sb.append(wt)
                    w2_sb = []
                    w2_r = r3(w2_d)
                    for dtl in range(16):
                        wt = wp.tile([128, E], F16, name=f"{tag}w2_{dtl}")
                        wdma(wt[:], w2_r[:, dtl, :])
                        w2_sb.append(wt)
                    x_new = []
                    for n in range(NB):
                        xin = x_cur[n]
                        accs = [pacc(f"{tag}acc{n}_{et}") for et in range(4)]
                        sds = []

                        def h2_emit(d):
                            for et in range(4):
                                nc.tensor.matmul(
                                    accs[et][:], w2_sb[d][:, ts(et, 128)],
                                    sds[d][:], start=(d == 0), stop=(d == 15))

                        for d in range(16):
                            hps = pwork(f"{tag}h1_{n}_{d}")
                            for et in range(4):
                                nc.tensor.matmul(
                                    hps[:], w1_sb[et][:, ts(d, 128)],
                                    xin[:, et, :],
                                    start=(et == 0), stop=(et == 3))
                            sg = wp.tile([128, 512], F32, tag="ffsg", bufs=3,
                                         name=f"{tag}sg{n}{d}")
                            nc.scalar.activation(sg[:], hps[:], Act.Sigmoid,
                                                 bias=b1m[:, d:d + 1])
                            sd = wp.tile([128, 512], F16, tag="ffsd", bufs=4,
                                         name=f"{tag}sd{n}{d}")
                            nc.vector.scalar_tensor_tensor(
                                sd[:], hps[:], b1[:, d:d + 1], sg[:],
                                op0=Alu.add, op1=Alu.mult)
                            sds.append(sd)
                            if d >= 1:
                                h2_emit(d - 1)
                        h2_emit(15)
                        xo = xtile(n, stage)
                        for et in range(4):
                            nc.vector.scalar_tensor_tensor(
                                xo[:, et, :], accs[et][:], b2[:, et:et + 1],
                                xin[:, et, :], op0=Alu.add, op1=Alu.add)
                        x_new.append(xo)
                    for n in range(NB):
                        x_cur[n] = x_new[n]

            ffn("ffm", w_ffm1_d, bf1_sb, bf1m_sb, w_ffm2_d, bf2_sb, 1)

            # p^T = pos_w @ pos_emb^T
            pT_sb = ppool.tile([128, 4, 1024], F16, tag="pT", name="pT_sb")
            for pf in range(4):
                for half in range(2):
                    ps = pwork(f"pps{pf}{half}")
                    for et in range(4):
                        nc.tensor.matmul(ps[:], wpos_sb[et][:, ts(pf, 128)],
                                         pos_sb[:, et, ts(half, 512)],
                                         start=(et == 0), stop=(et == 3))
                    nc.vector.tensor_copy(pT_sb[:, pf, ts(half, 512)], ps[:])
            pos_free()

            # ---- attention (head-pairs interleaved across batches) ----
            with tc.tile_pool(name="sattn", bufs=1) as wa:
                q_sb, k_sb, qv_sb, v_sb, oT_sb = [], [], [], [], []
                for n in range(NB):
                    x1 = x_cur[n]
                    q_ = wa.tile([128, 4, 512], F16, tag=f"q{n}",
                                 name=f"q_{n}")
                    k_ = wa.tile([128, 4, 512], F16, tag=f"k{n}",
                                 name=f"k_{n}")
                    qv_ = wa.tile([128, 4, 512], F16, tag=f"qv{n}",
                                  name=f"qv_{n}")
                    v_ = wa.tile([128, 4, 512], F16, tag=f"v{n}",
                                 name=f"v_{n}")
                    for i in range(4):
                        qps = pwork(f"qps{n}{i}")
                        for et in range(4):
                            nc.tensor.matmul(qps[:], wq_sb[et][:, ts(i, 128)],
                                             x1[:, et, :],
                                             start=(et == 0), stop=(et == 3))
                        nc.scalar.activation(q_[:, i, :], qps[:],
                                             Act.Identity,
                                             bias=bq_sb[:, i:i + 1])
                        kps = pwork(f"kps{n}{i}")
                        for et in range(4):
                            nc.tensor.matmul(kps[:], wk_sb[et][:, ts(i, 128)],
                                             x1[:, et, :],
                                             start=(et == 0), stop=(et == 3))
                        nc.scalar.activation(k_[:, i, :], kps[:],
                                             Act.Identity,
                                             bias=bk_sb[:, i:i + 1])
                        nc.gpsimd.tensor_scalar_add(qv_[:, i, :],
                                                    q_[:, i, :],
                                                    dvu_sb[:, i:i + 1])
                    for tt in range(4):
                        vps = pwork(f"vps{n}{tt}")
                        for et in range(4):
                            nc.tensor.matmul(vps[:], x1[:, et, ts(tt, 128)],
                                             wv_sb[et][:], start=(et == 0),
                                             stop=False)
                        nc.tensor.matmul(vps[:], ones16_sb[:], bv_row_sb[:],
                                         start=False, stop=True)
                        nc.vector.tensor_copy(v_[:, tt, :], vps[:])
                    q_sb.append(q_)
                    k_sb.append(k_)
                    qv_sb.append(qv_)
                    v_sb.append(v_)
                    oT_sb.append(wa.tile([128, 4, 512], F16, tag=f"oT{n}",
                                         name=f"oT_{n}"))

                LAG = 6
                units = [(hp, n, hh, tt) for hp in range(4)
                         for n in range(NB) for hh in range(2)
                         for tt in range(4)]
                at_tiles = {}
                bdsh_t = {}
                av_pend = []

                def produce(u):
                    hp, n, hh, tt = u
                    h = 2 * hp + hh
                    pr = slice(hh * 64, hh * 64 + 64)
                    tpos = (hh * 64, 0)
                    w0 = 384 - tt * 128
                    bdA = pwork(f"bdA{n}{h}{tt}")
                    bdB = psum.tile([128, 128], F32, tag="work",
                                    bufs=4, name=f"bdB{n}{h}{tt}")
                    nc.tensor.matmul(
                        bdA[:], qv_sb[n][pr, hp, ts(tt, 128)],
                        pT_sb[pr, hp, w0:w0 + 512],
                        start=True, stop=True, tile_position=tpos)
                    nc.tensor.matmul(
                        bdB[:], qv_sb[n][pr, hp, ts(tt, 128)],
                        pT_sb[pr, hp, w0 + 512:w0 + 640],
                        start=True, stop=True, tile_position=tpos)
                    bd_sb = wa.tile([128, 640], F16, tag="bdsb",
                                    bufs=8, name=f"bdsb{n}{h}{tt}")
                    nc.vector.tensor_copy(bd_sb[:, 0:512], bdA[:])
                    nc.scalar.copy(bd_sb[:, 512:640], bdB[:])
                    nc.scalar.dma_start(bds_d[n][h][tt], bd_sb[:])
                    bdsh = wa.tile([128, 512], F16, tag="bdsh",
                                   bufs=8, name=f"bdsh{n}{h}{tt}")
                    nc.scalar.dma_start(bdsh[:], diag_ap(bds_d[n][h][tt]))
                    bdsh_t[u] = bdsh

                soft_pend = []

                def consume(u):
                    hp, n, hh, tt = u
                    h = 2 * hp + hh
                    pr = slice(hh * 64, hh * 64 + 64)
                    tpos = (hh * 64, 0)
                    acps = pacc(f"ac{n}{h}{tt}")
                    nc.tensor.matmul(
                        acps[:], q_sb[n][pr, hp, ts(tt, 128)],
                        k_sb[n][pr, hp, :],
                        start=True, stop=True, tile_position=tpos)
                    sc = wa.tile([128, 512], F32, tag="sc",
                                 bufs=6, name=f"sc{n}{h}{tt}")
                    nc.vector.tensor_add(sc[:], acps[:], bdsh_t.pop(u)[:])
                    e_t = wa.tile([128, 512], F16, tag="esb",
                                  bufs=8, name=f"e{n}{h}{tt}")
                    zz = wa.tile([128, 1], F32, tag="z", bufs=8,
                                 name=f"z{n}{h}{tt}")
                    nc.scalar.activation(e_t[:], sc[:], Act.Exp,
                                         accum_out=zz[:])
                    soft_pend.append((u, e_t, zz))
                    if len(soft_pend) > 1:
                        consume_b(*soft_pend.pop(0))

                a_hold = {}

                def consume_b(u, e_t, zz):
                    hp, n, hh, tt = u
                    h = 2 * hp + hh
                    rz = wa.tile([128, 1], F32, tag="rz", bufs=8,
                                 name=f"rz{n}{h}{tt}")
                    nc.vector.reciprocal(rz[:], zz[:])
                    a_t = wa.tile([128, 512], F16, tag="asb",
                                  bufs=10, name=f"a{n}{h}{tt}")
                    nc.gpsimd.tensor_scalar_mul(a_t[:], e_t[:], rz[:, 0:1])
                    a_hold.setdefault((n, hp, hh), []).append(a_t)
                    if tt == 3:
                        # all 4 xbar transposes of the head back-to-back:
                        # one xbar-mode round trip instead of four
                        at = wa.tile([128, 4, 512], F16, tag="at", bufs=6,
                                     name=f"at{n}{hp}{hh}")
                        at_tiles[(n, hp, hh)] = at
                        for tt_, at_src in enumerate(a_hold.pop((n, hp, hh))):
                            nc.sync.dma_start_transpose(
                                at[:, :, ts(tt_, 128)], at_src[:])
                        av_pend.append((hp, n, hh))

                def emit_av(key):
                    hp, n, hh = key
                    h = 2 * hp + hh
                    ats = at_tiles.pop((n, hp, hh))
                    ops_ = psum.tile([64, 512], F32, tag="acc",
                                     bufs=4, name=f"ops{n}{h}")
                    for st in range(4):
                        nc.tensor.matmul(
                            ops_[:], v_sb[n][:, st, h * 64:h * 64 + 64],
                            ats[:, st, :], start=(st == 0), stop=(st == 3))
                    nc.vector.tensor_copy(oT_sb[n][pr_of(hh), hp, :], ops_[:])
                    if hp == 3 and hh == 1:
                        oproj(n)

                def oproj(n):
                    x2 = xtile(n, 2)
                    for of in range(4):
                        pps = pwork(f"oproj{n}{of}")
                        for hp in range(4):
                            nc.tensor.matmul(pps[:], wo_sb[hp][:, ts(of, 128)],
                                             oT_sb[n][:, hp, :],
                                             start=(hp == 0), stop=(hp == 3))
                        nc.vector.scalar_tensor_tensor(
                            x2[:, of, :], pps[:], bo_sb[:, of:of + 1],
                            x_cur[n][:, of, :], op0=Alu.add, op1=Alu.add)
                    x_cur[n] = x2

                for i, u in enumerate(units):
                    produce(u)
                    if i >= LAG:
                        consume(units[i - LAG])
                        if len(av_pend) > 2:
                            emit_av(av_pend.pop(0))
                for i in range(len(units) - LAG, len(units)):
                    consume(units[i])
                    if len(av_pend) > 2:
                        emit_av(av_pend.pop(0))
                while soft_pend:
                    consume_b(*soft_pend.pop(0))
                    if len(av_pend) > 2:
                        emit_av(av_pend.pop(0))
                while av_pend:
                    emit_av(av_pend.pop(0))

            # ---- conv module ----
            with tc.tile_pool(name="sconv", bufs=1) as wc:
                wpw1_sb = []
                pw1_r = r3(w_pw1_d)
                for et in range(4):
                    wt = wc.tile([128, 2 * E], F16, name=f"wpw1_{et}")
                    nc.sync.dma_start(wt[:], pw1_r[:, et, :])
                    wpw1_sb.append(wt)
                dwdiag_sb = wc.tile([128, 4, KC, 128], F16, name="dwdiag_sb")
                nc.scalar.dma_start(
                    dwdiag_sb[:],
                    w_dwdiag_d.rearrange("p (c k j) -> p c k j", c=4, k=KC))
                wpw2_sb = []
                pw2_r = r3(w_pw2_d)
                for et in range(4):
                    wt = wc.tile([128, E], F16, name=f"wpw2_{et}")
                    nc.sync.dma_start(wt[:], pw2_r[:, et, :])
                    wpw2_sb.append(wt)
                for n in range(NB):
                    x2 = x_cur[n]
                    ys = []
                    for cf in range(4):
                        bps = pwork(f"glb{n}{cf}")
                        for et in range(4):
                            nc.tensor.matmul(bps[:],
                                             wpw1_sb[et][:, ts(cf + 4, 128)],
                                             x2[:, et, :],
                                             start=(et == 0), stop=(et == 3))
                        sgl = wc.tile([128, 512], F32, tag="cvsg", bufs=3,
                                      name=f"cvsg{n}{cf}")
                        nc.scalar.activation(sgl[:], bps[:], Act.Sigmoid,
                                             bias=bpb_sb[:, cf:cf + 1])
                        aps = pwork(f"gla{n}{cf}")
                        for et in range(4):
                            nc.tensor.matmul(aps[:],
                                             wpw1_sb[et][:, ts(cf, 128)],
                                             x2[:, et, :],
                                             start=(et == 0), stop=(et == 3))
                        glu = wc.tile([128, 542], F16, tag="glu", bufs=6,
                                      name=f"glu{n}{cf}")
                        nc.gpsimd.memset(glu[:, 0:PAD], 0.0)
                        nc.gpsimd.memset(glu[:, 527:542], 0.0)
                        nc.vector.scalar_tensor_tensor(
                            glu[:, PAD:527], aps[:], bpa_sb[:, cf:cf + 1],
                            sgl[:], op0=Alu.add, op1=Alu.mult)
                        # depthwise conv: 31 diagonal-matrix matmuls on PE
                        dwps = pacc(f"dwps{n}{cf}")
                        for k_ in range(KC):
                            nc.tensor.matmul(dwps[:], dwdiag_sb[:, cf, k_, :],
                                             glu[:, k_:k_ + 512],
                                             start=(k_ == 0),
                                             stop=(k_ == KC - 1))
                        sg2 = wc.tile([128, 512], F32, tag="cvsg", bufs=3,
                                      name=f"dwsg{n}{cf}")
                        nc.scalar.activation(sg2[:], dwps[:], Act.Sigmoid,
                                             bias=bdwm_sb[:, cf:cf + 1])
                        y_ = wc.tile([128, 512], F16, tag="ydw", bufs=5,
                                     name=f"ydw{n}{cf}")
                        nc.vector.scalar_tensor_tensor(
                            y_[:], dwps[:], bdw_sb[:, cf:cf + 1], sg2[:],
                            op0=Alu.add, op1=Alu.mult)
                        ys.append(y_)
                    x3 = xtile(n, 3)
                    for of in range(4):
                        cps = pacc(f"pw2{n}{of}")
                        for cf in range(4):
                            nc.tensor.matmul(cps[:],
                                             wpw2_sb[cf][:, ts(of, 128)],
                                             ys[cf][:],
                                             start=(cf == 0), stop=(cf == 3))
                        nc.vector.scalar_tensor_tensor(
                            x3[:, of, :], cps[:], bp2_sb[:, of:of + 1],
                            x2[:, of, :], op0=Alu.add, op1=Alu.add)
                    x_cur[n] = x3

            # ---- final FFN ----
            ffn("ff2", w_ff1_d, bg1_sb, bg1m_sb, w_ff2_d, bg2_sb, 4)

            # ---- BasicNorm + output ----
            yt_r = [r3(yt_d[n]) for n in range(NB)]
            with tc.tile_pool(name="nrm", bufs=1) as nrm:
                for n in range(NB):
                    x4 = x_cur[n]
                    msps = psum.tile([1, 512], F32, tag="work", bufs=4,
                                     name=f"ms{n}")
                    for et in range(4):
                        sq = nrm.tile([128, 512], F16, tag="sq", bufs=2,
                                      name=f"sq{n}{et}")
                        nc.scalar.activation(sq[:], x4[:, et, :], Act.Square)
                        nc.tensor.matmul(msps[:], onescol16_sb[:], sq[:],
                                         start=(et == 0), stop=(et == 3))
                    sc1 = nrm.tile([1, 512], F32, tag="sc1", bufs=2,
                                   name=f"sc1{n}")
                    nc.scalar.activation(sc1[:], msps[:], Act.Sqrt,
                                         bias=eps_sb[0:1, 0:1], scale=1.0 / E)
                    rsc = nrm.tile([1, 512], F32, tag="rsc", bufs=2,
                                   name=f"rsc{n}")
                    nc.vector.reciprocal(rsc[:], sc1[:])
                    rscr = nrm.tile([1, 512], F32R, tag="rscr", bufs=2,
                                    name=f"rscr{n}")
                    nc.vector.tensor_copy(rscr[:], rsc[:])
                    bcps = pacc(f"bc{n}")
                    nc.tensor.matmul(bcps[:], ones32r_sb[:], rscr[:],
                                     start=True, stop=True)
                    for et in range(4):
                        yo = nrm.tile([128, 512], F32, tag="yo", bufs=3,
                                      name=f"yo{n}{et}")
                        nc.vector.tensor_mul(yo[:], x4[:, et, :], bcps[:])
                        nc.sync.dma_start(yt_r[n][:, et, :], yo[:])

        for _rep in range(repeat):
            emit_rep()

        psum_ctx.__exit__(None, None, None)
        ppool_ctx.__exit__(None, None, None)
        xpool_ctx.__exit__(None, None, None)
        wts_ctx.__exit__(None, None, None)
        cpool_ctx.__exit__(None, None, None)

    nc.compile()
    return nc


def _prep_inputs(inputs):
    f32 = np.float32
    f16 = np.float16
    s = np.float32(D ** -0.5)
    src = np.asarray(inputs['src'], f32)
    pos_emb = np.asarray(inputs['pos_emb'], f32)
    ipw = np.asarray(inputs['in_proj_w'], f32)
    ipb = np.asarray(inputs['in_proj_b'], f32)
    bu = np.asarray(inputs['pos_bias_u'], f32).reshape(E)
    bv = np.asarray(inputs['pos_bias_v'], f32).reshape(E)

    def t_(a):
        return np.ascontiguousarray(np.asarray(a, f32).T.astype(f16))

    def btile(b):  # (F,) -> (128, F//128) with [p, i] = b[i*128+p]
        b = np.asarray(b, f32)
        return np.ascontiguousarray(b.reshape(-1, 128).T)

    pos_t = np.zeros((E, 1024), f16)
    pos_t[:, :2 * T - 1] = pos_emb[0].T.astype(f16)

    dw = np.asarray(inputs['conv_dw_w'], f32).reshape(E, KC)
    dwr = dw.reshape(4, 128, KC).transpose(1, 0, 2)      # (128p, 4cf, 31k)
    dwdiag = np.zeros((128, 4, KC, 128), f16)
    pidx = np.arange(128)
    dwdiag[pidx, :, :, pidx] = dwr.astype(f16)
    w_dwdiag = np.ascontiguousarray(dwdiag.reshape(128, 4 * KC * 128))

    common = {
        'pos_t': pos_t,
        'w_ffm1': t_(inputs['ffm_w1']), 'bf1': btile(inputs['ffm_b1']),
        'bf1m': btile(np.asarray(inputs['ffm_b1'], f32) - 1.0),
        'w_ffm2': t_(inputs['ffm_w2']), 'bf2': btile(inputs['ffm_b2']),
        'w_q': np.ascontiguousarray((ipw[0:E] * s).T.astype(f16)),
        'w_k': t_(ipw[E:2 * E]), 'w_v': t_(ipw[2 * E:3 * E]),
        'bq': btile(ipb[0:E] * s + bu), 'bk': btile(ipb[E:2 * E]),
        'dvu': btile(bv - bu),
        'bv_row': np.ascontiguousarray(
            ipb[2 * E:3 * E].reshape(1, E).astype(f16)),
        'w_pos': t_(inputs['pos_w']),
        'w_out': t_(inputs['out_w']), 'bo': btile(inputs['out_b']),
        'w_pw1': t_(inputs['conv_pw1_w']),
        'bpa': btile(np.asarray(inputs['conv_pw1_b'], f32)[0:E]),
        'bpb': btile(np.asarray(inputs['conv_pw1_b'], f32)[E:2 * E]),
        'w_dwdiag': w_dwdiag, 'bdw': btile(inputs['conv_dw_b']),
        'bdwm': btile(np.asarray(inputs['conv_dw_b'], f32) - 1.0),
        'w_pw2': t_(inputs['conv_pw2_w']), 'bp2': btile(inputs['conv_pw2_b']),
        'w_ff1': t_(inputs['ff_w1']), 'bg1': btile(inputs['ff_b1']),
        'bg1m': btile(np.asarray(inputs['ff_b1'], f32) - 1.0),
        'w_ff2': t_(inputs['ff_w2']), 'bg2': btile(inputs['ff_b2']),
        'eps_c': np.exp(np.asarray(inputs['norm_eps'], f32)).reshape(1, 1),
        'ones16': np.ones((1, 128), f16),
        'onescol16': np.ones((128, 1), f16),
        'ones32': np.ones((1, 128), f32),
        'ident16': np.eye(128, dtype=f16),
    }

    src_t = np.ascontiguousarray(src.transpose(1, 2, 0))  # (N, E, T)
    in_maps = []
    for c in range(NCORE):
        m = dict(common)
        m['xt'] = np.ascontiguousarray(
            src_t[NB * c:NB * (c + 1)].astype(f16))
        in_maps.append(m)
    return in_maps


def _run(inputs, trace=False):
    from concourse import bass_utils
    if 'nc1' not in _cached:
        _cached['nc1'] = _build()
    nc = _cached['nc1']
    in_maps = _prep_inputs(inputs)
    res = bass_utils.run_bass_kernel_spmd(nc, in_maps,
                                          core_ids=list(range(NCORE)),
                                          trace=trace)
    yts = np.stack([res.results[c]['yt'] for c in range(NCORE)])  # (8,2,E,T)
    out = np.ascontiguousarray(
        yts.transpose(3, 0, 1, 2).reshape(T, N, E)).astype(np.float32)
    return out, res


def kernel(**inputs):
    out, _ = _run(inputs, trace=False)
    return out


def _make_runner(inputs, repeat=1):
    """Build a zero-transfer on-device runner for timing.

    Mirrors bass2jax.run_bass_via_pjrt's shard_map setup but without buffer
    donation, so nothing is re-transferred between timed calls.
    """
    import jax
    import numpy as _np
    import concourse.mybir as mybir
    from concourse.bass2jax import (_bass_exec_p, install_neuronx_cc_hook,
                                    partition_id_tensor)
    from jax.experimental.shard_map import shard_map
    from jax.sharding import Mesh, PartitionSpec, NamedSharding

    key = f'nc{repeat}'
    if key not in _cached:
        _cached[key] = _build(repeat)
    nc = _cached[key]
    install_neuronx_cc_hook()
    in_maps = _prep_inputs(inputs)

    in_names, out_names, out_avals, zero_outs = [], [], [], []
    for alloc in nc.m.functions[0].allocations:
        if not isinstance(alloc, mybir.MemoryLocationSet):
            continue
        name = alloc.memorylocations[0].name
        if alloc.kind == "ExternalInput":
            if nc.partition_id_tensor is None or \
                    name != nc.partition_id_tensor.name:
                in_names.append(name)
        elif alloc.kind == "ExternalOutput":
            out_names.append(name)
            shape = tuple(alloc.tensor_shape)
            dtype = mybir.dt.np(alloc.dtype)
            out_avals.append(jax.core.ShapedArray(shape, dtype))
            zero_outs.append(_np.zeros(shape, dtype))
    n_params = len(in_names)
    all_names = in_names + out_names
    if nc.partition_id_tensor is not None:
        all_names = all_names + [nc.partition_id_tensor.name]

    def _body(*args):
        operands = list(args)
        if nc.partition_id_tensor is not None:
            operands.append(partition_id_tensor())
        outs = _bass_exec_p.bind(
            *operands, out_avals=tuple(out_avals), in_names=tuple(all_names),
            out_names=tuple(out_names), lowering_input_output_aliases=(),
            sim_require_finite=True, sim_require_nnan=True, nc=nc)
        return tuple(outs)

    devices = jax.devices()[:NCORE]
    mesh = Mesh(_np.asarray(devices), ("core",))
    spec = PartitionSpec("core")
    sharded = jax.jit(shard_map(
        _body, mesh=mesh, in_specs=(spec,) * (n_params + len(out_names)),
        out_specs=(spec,) * len(out_names), check_rep=False))
    sh = NamedSharding(mesh, spec)
    concat_in = [jax.device_put(
        _np.concatenate([_np.asarray(in_maps[c][nm]) for c in range(NCORE)],
                        axis=0), sh) for nm in in_names]
    concat_zero = [jax.device_put(
        _np.zeros((NCORE * z.shape[0], *z.shape[1:]), z.dtype), sh)
        for z in zero_outs]

    def run():
        out = sharded(*concat_in, *concat_zero)
        jax.block_until_ready(out)
        return out

    def gather(out):
        yts = _np.asarray(out[out_names.index('yt')]).reshape(
            NCORE, NB, E, T)
        return _np.ascontiguousarray(
            yts.transpose(3, 0, 1, 2).reshape(T, N, E)).astype(_np.float32)

    return run, gather


def _bench(inputs, iters=10, repeat=1):
    import time
    run, gather = _make_runner(inputs, repeat)
    out = run()
    times = []
    for _ in range(iters):
        t0 = time.perf_counter()
        out = run()
        times.append(time.perf_counter() - t0)
    return gather(out), times
